# revision 3
# baseline (speedup 1.0000x reference)
"""MoE layer (N=4096, D=1024, H=4096, E=8, top-2) on 8 Trainium2 cores.

Strategy (expert-parallel, per the sharding hint):
  - Host computes the tiny gate (x @ Wg + bg), top-2 expert ids and softmax
    weights, then dispatches each token's row to its experts' cores
    (the host-side shard step IS the all-to-all dispatch).
  - Core e holds expert e's weights and runs the FFN for the <=C tokens
    routed to it:  y = relu(x_e @ W1[e] + b1[e]) @ W2[e], scaled by the
    per-token gate weight on-device.
  - Host scatter-adds the per-expert outputs (plus the w*b2 term) into the
    full [N, D] output.

Device kernel (identical SPMD program on all 8 cores):
  - All matmuls run as float32r (~tf32 precision, bf16-rate on the PE).
  - Activations stay in "transposed" layout for gemm1 (hT: hidden on
    partitions, tokens on free axis) so both gemms consume natural layouts:
      gemm1: hT[h,t] += W1[dk,h].T @ xT[dk,t]   (stationary W1 tile)
      gemm2: y[t,d]  += hT[hk,t].T @ W2[hk,d]   (stationary hT tile)
  - H is processed in slabs of 512; W1/W2 slabs are double-buffered from
    HBM; y accumulates in SBUF across slabs; bias+relu fused on DVE.
"""

import numpy as np

from concourse import bacc
import concourse.mybir as mybir
from concourse.tile import TileContext
import concourse.bass_utils as bass_utils

N_TOK, D, H, E, TOPK = 4096, 1024, 4096, 8, 2
NCORES = 8
C = 1280  # per-expert token capacity (mean N*TOPK/E = 1024, sigma ~30)
SLAB = 512  # hidden-dim slab resident in SBUF
TOK_SLICES = [(0, 512), (512, 512), (1024, 256)]  # all >=256 for fp32r rate
assert sum(t[1] for t in TOK_SLICES) == C

# Run the bass kernel with tracing (set by test.py to profile); results of
# the last run are stashed in LAST_RESULTS for the harness to inspect.
TRACE = False
LAST_RESULTS = None

_NC_CACHE = {}


def _build_nc():
    f32, f32r = mybir.dt.float32, mybir.dt.float32r
    nc = bacc.Bacc("TRN2", target_bir_lowering=False)
    xT = nc.dram_tensor("xT", [D, C], f32r, kind="ExternalInput")
    W1 = nc.dram_tensor("W1", [D, H], f32r, kind="ExternalInput")
    W2 = nc.dram_tensor("W2", [H, D], f32r, kind="ExternalInput")
    b1 = nc.dram_tensor("b1", [H, 1], f32, kind="ExternalInput")
    wg = nc.dram_tensor("wg", [C, 1], f32, kind="ExternalInput")
    y = nc.dram_tensor("y", [C, D], f32, kind="ExternalOutput")

    n_slab = H // SLAB  # 8
    n_dk = D // 128  # 8
    n_hm = SLAB // 128  # 4
    n_tk = C // 128  # 10
    n_dn = D // 512  # 2
    add, mx = mybir.AluOpType.add, mybir.AluOpType.max

    with TileContext(nc) as tc:
        with (
            tc.tile_pool(name="xp", bufs=1) as xp,
            tc.tile_pool(name="w1p", bufs=2) as w1p,
            tc.tile_pool(name="w2p", bufs=2) as w2p,
            tc.tile_pool(name="hp", bufs=2) as hp,
            tc.tile_pool(name="yp", bufs=1) as yp,
            tc.tile_pool(name="cp", bufs=2) as cp,
            tc.tile_pool(name="ps1", bufs=4, space="PSUM") as ps1,
            tc.tile_pool(name="ps2", bufs=4, space="PSUM") as ps2,
        ):
            xt = []
            for dk in range(n_dk):
                t = xp.tile([128, C], f32r, tag=f"x{dk}")
                nc.sync.dma_start(out=t, in_=xT[dk * 128 : (dk + 1) * 128, :])
                xt.append(t)
            wgt = []
            for tk in range(n_tk):
                t = cp.tile([128, 1], f32, tag=f"wg{tk}")
                nc.sync.dma_start(out=t, in_=wg[tk * 128 : (tk + 1) * 128, :])
                wgt.append(t)
            yt = [
                yp.tile([128, D], f32, tag=f"y{tk}", name=f"y{tk}")
                for tk in range(n_tk)
            ]

            for s in range(n_slab):
                w1t = []
                for dk in range(n_dk):
                    t = w1p.tile([128, SLAB], f32r, tag=f"w1_{dk}")
                    nc.sync.dma_start(
                        out=t,
                        in_=W1[dk * 128 : (dk + 1) * 128, s * SLAB : (s + 1) * SLAB],
                    )
                    w1t.append(t)
                w2t = []
                for hk in range(n_hm):
                    t = w2p.tile([128, D], f32r, tag=f"w2_{hk}")
                    h0 = s * SLAB + hk * 128
                    nc.sync.dma_start(out=t, in_=W2[h0 : h0 + 128, :])
                    w2t.append(t)
                b1t = []
                for hm in range(n_hm):
                    t = cp.tile([128, 1], f32, tag=f"b1_{hm}")
                    h0 = s * SLAB + hm * 128
                    nc.sync.dma_start(out=t, in_=b1[h0 : h0 + 128, :])
                    b1t.append(t)

                # gemm1: hT slab = relu(W1s.T @ xT + b1)
                hts = []
                for hm in range(n_hm):
                    ht = hp.tile([128, C], f32r, tag=f"h{hm}")
                    for t0, tn in TOK_SLICES:
                        ps = ps1.tile([128, 512], f32, tag="ps1")
                        for dk in range(n_dk):
                            nc.tensor.matmul(
                                ps[:, :tn],
                                w1t[dk][:, hm * 128 : (hm + 1) * 128],
                                xt[dk][:, t0 : t0 + tn],
                                start=(dk == 0),
                                stop=(dk == n_dk - 1),
                            )
                        nc.vector.tensor_scalar(
                            ht[:, t0 : t0 + tn], ps[:, :tn], b1t[hm], 0.0, add, mx
                        )
                    hts.append(ht)

                # gemm2: y += hT_slab.T @ W2s
                for tk in range(n_tk):
                    for dn in range(n_dn):
                        ps = ps2.tile([128, 512], f32, tag="ps2")
                        for hk in range(n_hm):
                            nc.tensor.matmul(
                                ps,
                                hts[hk][:, tk * 128 : (tk + 1) * 128],
                                w2t[hk][:, dn * 512 : (dn + 1) * 512],
                                start=(hk == 0),
                                stop=(hk == n_hm - 1),
                            )
                        ys = yt[tk][:, dn * 512 : (dn + 1) * 512]
                        if s == 0:
                            nc.vector.tensor_copy(ys, ps)
                        else:
                            nc.vector.tensor_add(ys, ys, ps)

            for tk in range(n_tk):
                nc.vector.tensor_scalar_mul(yt[tk], yt[tk], wgt[tk])
                nc.sync.dma_start(out=y[tk * 128 : (tk + 1) * 128, :], in_=yt[tk])
    nc.compile()
    return nc


def _get_nc():
    if "nc" not in _NC_CACHE:
        _NC_CACHE["nc"] = _build_nc()
    return _NC_CACHE["nc"]


def kernel(x, Wg, bg, W1, b1, W2, b2):
    global LAST_RESULTS
    x = np.asarray(x, dtype=np.float32)
    Wg = np.asarray(Wg, dtype=np.float32)
    bg = np.asarray(bg, dtype=np.float32)
    W1 = np.asarray(W1, dtype=np.float32)
    b1 = np.asarray(b1, dtype=np.float32)
    W2 = np.asarray(W2, dtype=np.float32)
    b2 = np.asarray(b2, dtype=np.float32)

    # --- gate + top-k routing (replicated small gate, on host) ---
    g = x @ Wg + bg  # [N, E]
    order = np.argsort(-g, axis=1, kind="stable")[:, :TOPK]  # [N, 2]
    topv = np.take_along_axis(g, order, axis=1)
    topv = topv - topv.max(axis=1, keepdims=True)
    ex = np.exp(topv)
    sw = ex / ex.sum(axis=1, keepdims=True)  # [N, 2] softmax over selected

    nc = _get_nc()
    in_maps = []
    routing = []
    for e in range(E):
        tok, kk = np.where(order == e)
        cnt = tok.size
        assert cnt <= C, f"expert {e} overflow: {cnt} > {C}"
        xTe = np.zeros((D, C), np.float32)
        xTe[:, :cnt] = x[tok].T
        wge = np.zeros((C, 1), np.float32)
        wge[:cnt, 0] = sw[tok, kk]
        in_maps.append(
            {
                "xT": xTe,
                "W1": np.ascontiguousarray(W1[e]),
                "W2": np.ascontiguousarray(W2[e]),
                "b1": np.ascontiguousarray(b1[e].reshape(H, 1)),
                "wg": wge,
            }
        )
        routing.append((tok, kk, cnt))

    LAST_RESULTS = bass_utils.run_bass_kernel_spmd(
        nc, in_maps, core_ids=list(range(NCORES)), trace=TRACE
    )

    # --- combine: scatter-add weighted expert outputs ---
    out = np.zeros((N_TOK, D), np.float32)
    for e in range(E):
        tok, kk, cnt = routing[e]
        # token ids are unique within one expert's list, so += is safe
        out[tok] += LAST_RESULTS.results[e]["y"][:cnt]
        if np.any(b2[e]):
            out[tok] += sw[tok, kk][:, None] * b2[e][None, :]
    return out


# revision 4
# speedup vs baseline: 1.0923x; 1.0923x over previous
"""MoE layer (N=4096, D=1024, H=4096, E=8, top-2) on 8 Trainium2 cores.

Strategy (expert-parallel, per the sharding hint):
  - Host computes the tiny gate (x @ Wg + bg), top-2 expert ids and softmax
    weights, then dispatches each token's row to its experts' cores
    (the host-side shard step IS the all-to-all dispatch).
  - Core e holds expert e's weights and runs the FFN for the <=C tokens
    routed to it:  y_e = relu(x_e @ W1[e] + b1[e]) @ W2[e].
  - Host combines: out[tok] += w_tok * (y_e[tok] + b2[e])  (scatter-add).

Device kernel (identical SPMD program on all 8 cores):
  - All matmuls run as float32r (~tf32 precision, bf16-rate on the PE).
  - Activations stay "transposed" (hT: hidden on partitions, tokens on the
    free axis) so both gemms consume natural weight layouts:
      gemm1: hT[h,t] += W1[dk,h].T @ xT[dk,t]   (stationary W1 tile)
      gemm2: y[t,d]  += hT[hk,t].T @ W2[hk,d]   (stationary hT tile)
  - H is processed in slabs: W1 streams in 512-wide chunks (small first
    chunk -> PE starts early), W2 in 1024-wide slabs so y accumulates in
    SBUF with only 4 add passes. All weight pools single-buffered; loads
    hide under the opposite gemm of the pipeline.
"""

import numpy as np

from concourse import bacc
import concourse.mybir as mybir
from concourse.tile import TileContext
import concourse.bass_utils as bass_utils

N_TOK, D, H, E, TOPK = 4096, 1024, 4096, 8, 2
NCORES = 8
C = 1152  # per-expert token capacity (actual max count 1091, mean 1024)
TOK_SLICES = [(0, 384), (384, 384), (768, 384)]  # all >=256 for fp32r rate
SLAB1 = 512  # gemm1 (W1) hidden chunk
SLAB2 = 1024  # gemm2 (W2) hidden slab; y adds once per slab
assert sum(t[1] for t in TOK_SLICES) == C

TRACE = False
TRACE_CORES = None
LAST_RESULTS = None

_NC_CACHE = {}


def _build_nc():
    f32, f32r = mybir.dt.float32, mybir.dt.float32r
    nc = bacc.Bacc("TRN2", target_bir_lowering=False)
    xT = nc.dram_tensor("xT", [D, C], f32r, kind="ExternalInput")
    W1 = nc.dram_tensor("W1", [D, H], f32r, kind="ExternalInput")
    W2 = nc.dram_tensor("W2", [H, D], f32r, kind="ExternalInput")
    b1 = nc.dram_tensor("b1", [H, 1], f32, kind="ExternalInput")
    y = nc.dram_tensor("y", [C, D], f32, kind="ExternalOutput")

    n_dk = D // 128  # 8
    n_s2 = H // SLAB2  # 4 gemm2 slabs
    n_half = SLAB2 // SLAB1  # 2 gemm1 chunks per gemm2 slab
    n_hm = SLAB1 // 128  # 4
    n_hk = SLAB2 // 128  # 8
    n_tk = C // 128  # 9
    n_dn = D // 512  # 2
    add, mx = mybir.AluOpType.add, mybir.AluOpType.max

    with TileContext(nc) as tc:
        with (
            tc.tile_pool(name="xp", bufs=1) as xp,
            tc.tile_pool(name="w1p", bufs=2) as w1p,
            tc.tile_pool(name="w2p", bufs=1) as w2p,
            tc.tile_pool(name="hp", bufs=1) as hp,
            tc.tile_pool(name="yp", bufs=1) as yp,
            tc.tile_pool(name="cp", bufs=2) as cp,
            tc.tile_pool(name="ps1", bufs=4, space="PSUM") as ps1,
            tc.tile_pool(name="ps2", bufs=4, space="PSUM") as ps2,
        ):
            # --- startup: first W1 chunk + xT token-slice 0 first ---
            w1t = []
            for dk in range(n_dk):
                t = w1p.tile([128, SLAB1], f32r, tag=f"w1_{dk}", name=f"w1t{dk}")
                nc.sync.dma_start(out=t, in_=W1[dk * 128 : (dk + 1) * 128, 0:SLAB1])
                w1t.append(t)
            xt = []
            for dk in range(n_dk):
                t = xp.tile([128, C], f32r, tag=f"x{dk}", name=f"xt{dk}")
                t0, tn = TOK_SLICES[0]
                nc.sync.dma_start(
                    out=t[:, t0 : t0 + tn],
                    in_=xT[dk * 128 : (dk + 1) * 128, t0 : t0 + tn],
                )
                xt.append(t)
            for t0, tn in TOK_SLICES[1:]:
                for dk in range(n_dk):
                    nc.sync.dma_start(
                        out=xt[dk][:, t0 : t0 + tn],
                        in_=xT[dk * 128 : (dk + 1) * 128, t0 : t0 + tn],
                    )

            yt = [
                yp.tile([128, D], f32, tag=f"y{tk}", name=f"y{tk}")
                for tk in range(n_tk)
            ]

            for s2 in range(n_s2):
                # W2 slab loads (emitted before gemm1 work of this slab; with
                # bufs=1 they wait on last slab's gemm2 and hide under gemm1)
                w2t = []
                for hk in range(n_hk):
                    t = w2p.tile([128, D], f32r, tag=f"w2_{hk}", name=f"w2t{hk}")
                    h0 = s2 * SLAB2 + hk * 128
                    nc.sync.dma_start(out=t, in_=W2[h0 : h0 + 128, :])
                    w2t.append(t)

                hts = []
                for half in range(n_half):
                    s1 = s2 * n_half + half
                    if s1 > 0:  # chunk 0 loaded in the preamble
                        w1t = []
                        for dk in range(n_dk):
                            t = w1p.tile(
                                [128, SLAB1], f32r, tag=f"w1_{dk}", name=f"w1t{dk}"
                            )
                            h0 = s1 * SLAB1
                            nc.sync.dma_start(
                                out=t, in_=W1[dk * 128 : (dk + 1) * 128, h0 : h0 + SLAB1]
                            )
                            w1t.append(t)
                    b1t = []
                    for hm in range(n_hm):
                        t = cp.tile([128, 1], f32, tag=f"b1_{hm}", name=f"b1t{hm}")
                        h0 = s1 * SLAB1 + hm * 128
                        nc.sync.dma_start(out=t, in_=b1[h0 : h0 + 128, :])
                        b1t.append(t)

                    hts_half = [
                        hp.tile([128, C], f32r, tag=f"h{half}_{hm}", name=f"ht{hm}")
                        for hm in range(n_hm)
                    ]
                    # token-slice outer so the PE can start on slice 0 while
                    # later xT slices are still loading (first chunk only)
                    for t0, tn in TOK_SLICES:
                        for hm in range(n_hm):
                            ps = ps1.tile([128, 384], f32, tag="ps1", name="ps1t")
                            for dk in range(n_dk):
                                nc.tensor.matmul(
                                    ps[:, :tn],
                                    w1t[dk][:, hm * 128 : (hm + 1) * 128],
                                    xt[dk][:, t0 : t0 + tn],
                                    start=(dk == 0),
                                    stop=(dk == n_dk - 1),
                                )
                            nc.vector.tensor_scalar(
                                hts_half[hm][:, t0 : t0 + tn],
                                ps[:, :tn],
                                b1t[hm],
                                0.0,
                                add,
                                mx,
                            )
                    hts.extend(hts_half)

                # gemm2: y(+)= hT_slab.T @ W2s
                for tk in range(n_tk):
                    for dn in range(n_dn):
                        ps = ps2.tile([128, 512], f32, tag="ps2", name="ps2t")
                        for hk in range(n_hk):
                            nc.tensor.matmul(
                                ps,
                                hts[hk][:, tk * 128 : (tk + 1) * 128],
                                w2t[hk][:, dn * 512 : (dn + 1) * 512],
                                start=(hk == 0),
                                stop=(hk == n_hk - 1),
                            )
                        ys = yt[tk][:, dn * 512 : (dn + 1) * 512]
                        if s2 == 0:
                            nc.vector.tensor_copy(ys, ps)
                        else:
                            nc.vector.tensor_add(ys, ys, ps)
                    if s2 == n_s2 - 1:
                        nc.sync.dma_start(
                            out=y[tk * 128 : (tk + 1) * 128, :], in_=yt[tk]
                        )
    nc.compile()
    return nc


def _get_nc():
    if "nc" not in _NC_CACHE:
        _NC_CACHE["nc"] = _build_nc()
    return _NC_CACHE["nc"]


def kernel(x, Wg, bg, W1, b1, W2, b2):
    global LAST_RESULTS
    x = np.asarray(x, dtype=np.float32)
    Wg = np.asarray(Wg, dtype=np.float32)
    bg = np.asarray(bg, dtype=np.float32)
    W1 = np.asarray(W1, dtype=np.float32)
    b1 = np.asarray(b1, dtype=np.float32)
    W2 = np.asarray(W2, dtype=np.float32)
    b2 = np.asarray(b2, dtype=np.float32)

    # --- gate + top-k routing (replicated small gate, on host) ---
    g = x @ Wg + bg  # [N, E]
    order = np.argsort(-g, axis=1, kind="stable")[:, :TOPK]  # [N, 2]
    topv = np.take_along_axis(g, order, axis=1)
    topv = topv - topv.max(axis=1, keepdims=True)
    ex = np.exp(topv)
    sw = ex / ex.sum(axis=1, keepdims=True)  # [N, 2] softmax over selected

    nc = _get_nc()
    in_maps = []
    routing = []
    for e in range(E):
        tok, kk = np.where(order == e)
        cnt = tok.size
        assert cnt <= C, f"expert {e} overflow: {cnt} > {C}"
        xTe = np.zeros((D, C), np.float32)
        xTe[:, :cnt] = x[tok].T
        in_maps.append(
            {
                "xT": xTe,
                "W1": np.ascontiguousarray(W1[e]),
                "W2": np.ascontiguousarray(W2[e]),
                "b1": np.ascontiguousarray(b1[e].reshape(H, 1)),
            }
        )
        routing.append((tok, kk, cnt))

    kwargs = {}
    if TRACE_CORES is not None:
        kwargs["trace_cores"] = TRACE_CORES
    LAST_RESULTS = bass_utils.run_bass_kernel_spmd(
        nc, in_maps, core_ids=list(range(NCORES)), trace=TRACE, **kwargs
    )

    # --- combine: scatter-add gate-weighted expert outputs ---
    out = np.zeros((N_TOK, D), np.float32)
    for e in range(E):
        tok, kk, cnt = routing[e]
        ye = LAST_RESULTS.results[e]["y"][:cnt]
        if np.any(b2[e]):
            ye = ye + b2[e][None, :]
        # token ids are unique within one expert's list, so += is safe
        out[tok] += sw[tok, kk][:, None] * ye
    return out


# revision 8
# speedup vs baseline: 1.1441x; 1.0474x over previous
"""MoE layer (N=4096, D=1024, H=4096, E=8, top-2) on 8 Trainium2 cores.

Strategy (expert-parallel, per the sharding hint):
  - Host computes the tiny gate (x @ Wg + bg), top-2 expert ids and softmax
    weights, then dispatches each token's row to its experts' cores
    (the host-side shard step IS the all-to-all dispatch).
  - Core e holds expert e's weights and runs the FFN for the <=C tokens
    routed to it:  y_e = relu(x_e @ W1[e] + b1[e]) @ W2[e].
  - Host combines: out[tok] += w_tok * (y_e[tok] + b2[e])  (scatter-add).

Device kernel (identical SPMD program on all 8 cores):
  - All matmuls run as float32r (~tf32 precision, bf16-rate on the PE).
  - Activations stay "transposed" (hT: hidden on partitions, tokens on the
    free axis) so both gemms consume natural weight layouts:
      gemm1: hT[h,t] += W1[dk,h].T @ xT[dk,t]   (stationary W1 tile)
      gemm2: y[t,d]  += hT[hk,t].T @ W2[hk,d]   (stationary hT tile)
  - H is processed in slabs: W1 streams in 512-wide chunks (small first
    chunk -> PE starts early), W2 in 1024-wide slabs so y accumulates in
    SBUF with only 4 add passes. All weight pools single-buffered; loads
    hide under the opposite gemm of the pipeline.
"""

import numpy as np

from concourse import bacc
import concourse.mybir as mybir
from concourse.tile import TileContext
import concourse.bass_utils as bass_utils

N_TOK, D, H, E, TOPK = 4096, 1024, 4096, 8, 2
NCORES = 8
C = 1152  # per-expert token capacity (actual max count 1091, mean 1024)
TOK_SLICES = [(0, 384), (384, 384), (768, 384)]  # all >=256 for fp32r rate
SLAB1 = 512  # gemm1 (W1) hidden chunk
SLAB2 = 1024  # gemm2 (W2) hidden slab; y adds once per slab
assert sum(t[1] for t in TOK_SLICES) == C

TRACE = False
TRACE_CORES = None
LAST_RESULTS = None

_NC_CACHE = {}


def _build_nc():
    f32, f32r = mybir.dt.float32, mybir.dt.float32r
    nc = bacc.Bacc("TRN2", target_bir_lowering=False)
    xT = nc.dram_tensor("xT", [D, C], f32r, kind="ExternalInput")
    W1 = nc.dram_tensor("W1", [D, H], f32r, kind="ExternalInput")
    W2 = nc.dram_tensor("W2", [H, D], f32r, kind="ExternalInput")
    b1 = nc.dram_tensor("b1", [H, 1], f32, kind="ExternalInput")
    y = nc.dram_tensor("y", [C, D], f32, kind="ExternalOutput")

    n_dk = D // 128  # 8
    n_s2 = H // SLAB2  # 4 gemm2 slabs
    n_half = SLAB2 // SLAB1  # 2 gemm1 chunks per gemm2 slab
    n_hm = SLAB1 // 128  # 4
    n_hk = SLAB2 // 128  # 8
    n_tk = C // 128  # 9
    n_dn = D // 512  # 2
    add, mx = mybir.AluOpType.add, mybir.AluOpType.max

    with TileContext(nc) as tc:
        with (
            tc.tile_pool(name="xp", bufs=1) as xp,
            tc.tile_pool(name="w1p", bufs=2) as w1p,
            tc.tile_pool(name="w2p", bufs=1) as w2p,
            tc.tile_pool(name="hp", bufs=1) as hp,
            tc.tile_pool(name="yp", bufs=1) as yp,
            tc.tile_pool(name="cp", bufs=2) as cp,
            tc.tile_pool(name="ps1", bufs=4, space="PSUM") as ps1,
            tc.tile_pool(name="ps2", bufs=4, space="PSUM") as ps2,
        ):
            # --- HAM warmup: dummy matmuls on a zeroed tile run during the
            # initial weight/activation DMA wait so the PE clock-gate is
            # already released (2.4 GHz) when real work arrives ---
            warm = xp.tile([128, 512], mybir.dt.bfloat16, name="warm")
            nc.vector.memset(warm, 0.0)
            wps = ps1.tile([128, 384], f32, tag="ps1", name="warmps")
            for i in range(24):
                nc.tensor.matmul(
                    wps, warm[:, :128], warm[:, :384], start=(i == 0), stop=(i == 23)
                )

            # --- startup: first W1 chunk + xT token-slice 0 first ---
            w1t = []
            for dk in range(n_dk):
                t = w1p.tile([128, SLAB1], f32r, tag=f"w1_{dk}", name=f"w1t{dk}")
                nc.sync.dma_start(out=t, in_=W1[dk * 128 : (dk + 1) * 128, 0:SLAB1])
                w1t.append(t)
            xt = []
            for dk in range(n_dk):
                t = xp.tile([128, C], f32r, tag=f"x{dk}", name=f"xt{dk}")
                t0, tn = TOK_SLICES[0]
                nc.sync.dma_start(
                    out=t[:, t0 : t0 + tn],
                    in_=xT[dk * 128 : (dk + 1) * 128, t0 : t0 + tn],
                )
                xt.append(t)
            for t0, tn in TOK_SLICES[1:]:
                for dk in range(n_dk):
                    nc.sync.dma_start(
                        out=xt[dk][:, t0 : t0 + tn],
                        in_=xT[dk * 128 : (dk + 1) * 128, t0 : t0 + tn],
                    )

            yt = [
                yp.tile([128, D], f32, tag=f"y{tk}", name=f"y{tk}")
                for tk in range(n_tk)
            ]

            for s2 in range(n_s2):
                hts = []
                for half in range(n_half):
                    s1 = s2 * n_half + half
                    if s1 > 0:  # chunk 0 loaded in the preamble
                        w1t = []
                        for dk in range(n_dk):
                            t = w1p.tile(
                                [128, SLAB1], f32r, tag=f"w1_{dk}", name=f"w1t{dk}"
                            )
                            h0 = s1 * SLAB1
                            nc.sync.dma_start(
                                out=t, in_=W1[dk * 128 : (dk + 1) * 128, h0 : h0 + SLAB1]
                            )
                            w1t.append(t)
                    b1t = []
                    for hm in range(n_hm):
                        t = cp.tile([128, 1], f32, tag=f"b1_{hm}", name=f"b1t{hm}")
                        h0 = s1 * SLAB1 + hm * 128
                        nc.sync.dma_start(out=t, in_=b1[h0 : h0 + 128, :])
                        b1t.append(t)

                    hts_half = [
                        hp.tile([128, C], f32r, tag=f"h{half}_{hm}", name=f"ht{hm}")
                        for hm in range(n_hm)
                    ]
                    # token-slice outer so the PE can start on slice 0 while
                    # later xT slices are still loading (first chunk only)
                    for t0, tn in TOK_SLICES:
                        for hm in range(n_hm):
                            ps = ps1.tile([128, 384], f32, tag="ps1", name="ps1t")
                            for dk in range(n_dk):
                                nc.tensor.matmul(
                                    ps[:, :tn],
                                    w1t[dk][:, hm * 128 : (hm + 1) * 128],
                                    xt[dk][:, t0 : t0 + tn],
                                    start=(dk == 0),
                                    stop=(dk == n_dk - 1),
                                )
                            nc.vector.tensor_scalar(
                                hts_half[hm][:, t0 : t0 + tn],
                                ps[:, :tn],
                                b1t[hm],
                                0.0,
                                add,
                                mx,
                            )
                    hts.extend(hts_half)

                # W2 slab loads: emitted after the W1 chunk loads so they
                # queue behind them at startup (W1 is needed sooner); in
                # steady state the bufs=1 WAR on last slab's gemm2 gates the
                # start anyway and the load hides under this slab's gemm1.
                w2t = []
                for hk in range(n_hk):
                    t = w2p.tile([128, D], f32r, tag=f"w2_{hk}", name=f"w2t{hk}")
                    h0 = s2 * SLAB2 + hk * 128
                    nc.sync.dma_start(out=t, in_=W2[h0 : h0 + 128, :])
                    w2t.append(t)

                # gemm2: y(+)= hT_slab.T @ W2s
                for tk in range(n_tk):
                    for dn in range(n_dn):
                        ps = ps2.tile([128, 512], f32, tag="ps2", name="ps2t")
                        for hk in range(n_hk):
                            nc.tensor.matmul(
                                ps,
                                hts[hk][:, tk * 128 : (tk + 1) * 128],
                                w2t[hk][:, dn * 512 : (dn + 1) * 512],
                                start=(hk == 0),
                                stop=(hk == n_hk - 1),
                            )
                        ys = yt[tk][:, dn * 512 : (dn + 1) * 512]
                        if s2 == 0:
                            nc.vector.tensor_copy(ys, ps)
                        else:
                            nc.vector.tensor_add(ys, ys, ps)
                    if s2 == n_s2 - 1:
                        nc.sync.dma_start(
                            out=y[tk * 128 : (tk + 1) * 128, :], in_=yt[tk]
                        )
    nc.compile()
    return nc


def _get_nc():
    if "nc" not in _NC_CACHE:
        _NC_CACHE["nc"] = _build_nc()
    return _NC_CACHE["nc"]


def kernel(x, Wg, bg, W1, b1, W2, b2):
    global LAST_RESULTS
    x = np.asarray(x, dtype=np.float32)
    Wg = np.asarray(Wg, dtype=np.float32)
    bg = np.asarray(bg, dtype=np.float32)
    W1 = np.asarray(W1, dtype=np.float32)
    b1 = np.asarray(b1, dtype=np.float32)
    W2 = np.asarray(W2, dtype=np.float32)
    b2 = np.asarray(b2, dtype=np.float32)

    # --- gate + top-k routing (replicated small gate, on host) ---
    g = x @ Wg + bg  # [N, E]
    order = np.argsort(-g, axis=1, kind="stable")[:, :TOPK]  # [N, 2]
    topv = np.take_along_axis(g, order, axis=1)
    topv = topv - topv.max(axis=1, keepdims=True)
    ex = np.exp(topv)
    sw = ex / ex.sum(axis=1, keepdims=True)  # [N, 2] softmax over selected

    nc = _get_nc()
    in_maps = []
    routing = []
    for e in range(E):
        tok, kk = np.where(order == e)
        cnt = tok.size
        assert cnt <= C, f"expert {e} overflow: {cnt} > {C}"
        xTe = np.zeros((D, C), np.float32)
        xTe[:, :cnt] = x[tok].T
        in_maps.append(
            {
                "xT": xTe,
                "W1": np.ascontiguousarray(W1[e]),
                "W2": np.ascontiguousarray(W2[e]),
                "b1": np.ascontiguousarray(b1[e].reshape(H, 1)),
            }
        )
        routing.append((tok, kk, cnt))

    kwargs = {}
    if TRACE_CORES is not None:
        kwargs["trace_cores"] = TRACE_CORES
    LAST_RESULTS = bass_utils.run_bass_kernel_spmd(
        nc, in_maps, core_ids=list(range(NCORES)), trace=TRACE, **kwargs
    )

    # --- combine: scatter-add gate-weighted expert outputs ---
    out = np.zeros((N_TOK, D), np.float32)
    for e in range(E):
        tok, kk, cnt = routing[e]
        ye = LAST_RESULTS.results[e]["y"][:cnt]
        if np.any(b2[e]):
            ye = ye + b2[e][None, :]
        # token ids are unique within one expert's list, so += is safe
        out[tok] += sw[tok, kk][:, None] * ye
    return out


# revision 11
# speedup vs baseline: 1.1741x; 1.0262x over previous
"""MoE layer (N=4096, D=1024, H=4096, E=8, top-2) on 8 Trainium2 cores.

Strategy (expert-parallel, per the sharding hint):
  - Host computes the tiny gate (x @ Wg + bg), top-2 expert ids and softmax
    weights, then dispatches each token's row to its experts' cores
    (the host-side shard step IS the all-to-all dispatch).
  - Core e holds expert e's weights and runs the FFN for the <=C tokens
    routed to it:  y_e = relu(x_e @ W1[e] + b1[e]) @ W2[e].
  - Host combines: out[tok] += w_tok * (y_e[tok] + b2[e])  (scatter-add).

Device kernel (identical SPMD program on all 8 cores):
  - All matmuls run as float32r (~tf32 precision, bf16-rate on the PE).
  - Activations stay "transposed" (hT: hidden on partitions, tokens on the
    free axis) so both gemms consume natural weight layouts:
      gemm1: hT[h,t] += W1[dk,h].T @ xT[dk,t]   (stationary W1 tile)
      gemm2: y[t,d]  += hT[hk,t].T @ W2[hk,d]   (stationary hT tile)
  - H is processed in slabs: W1 streams in 512-wide chunks (small first
    chunk -> PE starts early), W2 in 1024-wide slabs so y accumulates in
    SBUF with only 4 add passes. All weight pools single-buffered; loads
    hide under the opposite gemm of the pipeline.
"""

import numpy as np

from concourse import bacc
import concourse.mybir as mybir
from concourse.tile import TileContext
import concourse.bass_utils as bass_utils

N_TOK, D, H, E, TOPK = 4096, 1024, 4096, 8, 2
NCORES = 8
C = 1152  # per-expert token capacity; must be divisible by 128 (max count 1091)
TOK_SLICES = [(0, 384), (384, 384), (768, 384)]  # all >=256 for fp32r rate
SLAB1 = 512  # gemm1 (W1) hidden chunk
SLAB2 = 1024  # gemm2 (W2) hidden slab; y adds once per slab
assert sum(t[1] for t in TOK_SLICES) == C

TRACE = False
TRACE_CORES = None
LAST_RESULTS = None

_NC_CACHE = {}


def _build_nc():
    f32, f32r = mybir.dt.float32, mybir.dt.float32r
    nc = bacc.Bacc("TRN2", target_bir_lowering=False)
    xT = nc.dram_tensor("xT", [D, C], f32r, kind="ExternalInput")
    W1 = nc.dram_tensor("W1", [D, H], f32r, kind="ExternalInput")
    W2 = nc.dram_tensor("W2", [H, D], f32r, kind="ExternalInput")
    b1 = nc.dram_tensor("b1", [H, 1], f32, kind="ExternalInput")
    y = nc.dram_tensor("y", [C, D], f32, kind="ExternalOutput")

    n_dk = D // 128  # 8
    n_s2 = H // SLAB2  # 4 gemm2 slabs
    n_half = SLAB2 // SLAB1  # 2 gemm1 chunks per gemm2 slab
    n_hm = SLAB1 // 128  # 4
    n_hk = SLAB2 // 128  # 8
    n_tk = C // 128  # 9
    n_dn = D // 512  # 2
    add, mx = mybir.AluOpType.add, mybir.AluOpType.max

    with TileContext(nc) as tc:
        with (
            tc.tile_pool(name="xp", bufs=1) as xp,
            tc.tile_pool(name="w1p", bufs=2) as w1p,
            tc.tile_pool(name="w2p", bufs=1) as w2p,
            tc.tile_pool(name="hp", bufs=1) as hp,
            tc.tile_pool(name="yp", bufs=1) as yp,
            tc.tile_pool(name="cp", bufs=2) as cp,
            tc.tile_pool(name="ps1", bufs=4, space="PSUM") as ps1,
            tc.tile_pool(name="ps2", bufs=4, space="PSUM") as ps2,
        ):
            _dma_i = [0]

            def hwdma(**kw):
                eng = (nc.sync, nc.scalar)[_dma_i[0] % 2]
                _dma_i[0] += 1
                eng.dma_start(**kw)

            # --- HAM warmup: dummy matmuls on a zeroed tile run during the
            # initial weight/activation DMA wait so the PE clock-gate is
            # already released (2.4 GHz) when real work arrives ---
            warm = xp.tile([128, 512], mybir.dt.bfloat16, name="warm")
            nc.vector.memset(warm, 0.0)
            wps = ps1.tile([128, 384], f32, tag="ps1", name="warmps")
            for i in range(36):
                nc.tensor.matmul(
                    wps, warm[:, :128], warm[:, :384], start=(i == 0), stop=(i == 35)
                )

            # --- startup: first W1 chunk + xT token-slice 0 first ---
            w1t = []
            for dk in range(n_dk):
                t = w1p.tile([128, SLAB1], f32r, tag=f"w1_{dk}", name=f"w1t{dk}")
                hwdma(out=t, in_=W1[dk * 128 : (dk + 1) * 128, 0:SLAB1])
                w1t.append(t)
            xt = []
            for dk in range(n_dk):
                t = xp.tile([128, C], f32r, tag=f"x{dk}", name=f"xt{dk}")
                t0, tn = TOK_SLICES[0]
                hwdma(
                    out=t[:, t0 : t0 + tn],
                    in_=xT[dk * 128 : (dk + 1) * 128, t0 : t0 + tn],
                )
                xt.append(t)
            for t0, tn in TOK_SLICES[1:]:
                for dk in range(n_dk):
                    hwdma(
                        out=xt[dk][:, t0 : t0 + tn],
                        in_=xT[dk * 128 : (dk + 1) * 128, t0 : t0 + tn],
                    )

            yt = [
                yp.tile([128, D], f32, tag=f"y{tk}", name=f"y{tk}")
                for tk in range(n_tk)
            ]

            for s2 in range(n_s2):
                hts = []
                for half in range(n_half):
                    s1 = s2 * n_half + half
                    if s1 > 0:  # chunk 0 loaded in the preamble
                        w1t = []
                        for dk in range(n_dk):
                            t = w1p.tile(
                                [128, SLAB1], f32r, tag=f"w1_{dk}", name=f"w1t{dk}"
                            )
                            h0 = s1 * SLAB1
                            hwdma(
                                out=t, in_=W1[dk * 128 : (dk + 1) * 128, h0 : h0 + SLAB1]
                            )
                            w1t.append(t)
                    b1t = []
                    for hm in range(n_hm):
                        t = cp.tile([128, 1], f32, tag=f"b1_{hm}", name=f"b1t{hm}")
                        h0 = s1 * SLAB1 + hm * 128
                        nc.gpsimd.dma_start(out=t, in_=b1[h0 : h0 + 128, :])
                        b1t.append(t)

                    hts_half = [
                        hp.tile([128, C], f32r, tag=f"h{half}_{hm}", name=f"ht{hm}")
                        for hm in range(n_hm)
                    ]
                    # token-slice outer so the PE can start on slice 0 while
                    # later xT slices are still loading (first chunk only)
                    for t0, tn in TOK_SLICES:
                        for hm in range(n_hm):
                            ps = ps1.tile([128, 384], f32, tag="ps1", name="ps1t")
                            for dk in range(n_dk):
                                nc.tensor.matmul(
                                    ps[:, :tn],
                                    w1t[dk][:, hm * 128 : (hm + 1) * 128],
                                    xt[dk][:, t0 : t0 + tn],
                                    start=(dk == 0),
                                    stop=(dk == n_dk - 1),
                                )
                            nc.vector.tensor_scalar(
                                hts_half[hm][:, t0 : t0 + tn],
                                ps[:, :tn],
                                b1t[hm],
                                0.0,
                                add,
                                mx,
                            )
                    hts.extend(hts_half)

                # W2 slab loads: emitted after the W1 chunk loads so they
                # queue behind them at startup (W1 is needed sooner); in
                # steady state the bufs=1 WAR on last slab's gemm2 gates the
                # start anyway and the load hides under this slab's gemm1.
                w2t = []
                for hk in range(n_hk):
                    t = w2p.tile([128, D], f32r, tag=f"w2_{hk}", name=f"w2t{hk}")
                    h0 = s2 * SLAB2 + hk * 128
                    hwdma(out=t, in_=W2[h0 : h0 + 128, :])
                    w2t.append(t)

                # gemm2: y(+)= hT_slab.T @ W2s
                for tk in range(n_tk):
                    for dn in range(n_dn):
                        ps = ps2.tile([128, 512], f32, tag="ps2", name="ps2t")
                        for hk in range(n_hk):
                            nc.tensor.matmul(
                                ps,
                                hts[hk][:, tk * 128 : (tk + 1) * 128],
                                w2t[hk][:, dn * 512 : (dn + 1) * 512],
                                start=(hk == 0),
                                stop=(hk == n_hk - 1),
                            )
                        ys = yt[tk][:, dn * 512 : (dn + 1) * 512]
                        if s2 == 0:
                            nc.vector.tensor_copy(ys, ps)
                        else:
                            nc.vector.tensor_add(ys, ys, ps)
                    if s2 == n_s2 - 1:
                        hwdma(out=y[tk * 128 : (tk + 1) * 128, :], in_=yt[tk])
    nc.compile()
    return nc


def _get_nc():
    if "nc" not in _NC_CACHE:
        _NC_CACHE["nc"] = _build_nc()
    return _NC_CACHE["nc"]


def kernel(x, Wg, bg, W1, b1, W2, b2):
    global LAST_RESULTS
    x = np.asarray(x, dtype=np.float32)
    Wg = np.asarray(Wg, dtype=np.float32)
    bg = np.asarray(bg, dtype=np.float32)
    W1 = np.asarray(W1, dtype=np.float32)
    b1 = np.asarray(b1, dtype=np.float32)
    W2 = np.asarray(W2, dtype=np.float32)
    b2 = np.asarray(b2, dtype=np.float32)

    # --- gate + top-k routing (replicated small gate, on host) ---
    g = x @ Wg + bg  # [N, E]
    order = np.argsort(-g, axis=1, kind="stable")[:, :TOPK]  # [N, 2]
    topv = np.take_along_axis(g, order, axis=1)
    topv = topv - topv.max(axis=1, keepdims=True)
    ex = np.exp(topv)
    sw = ex / ex.sum(axis=1, keepdims=True)  # [N, 2] softmax over selected

    nc = _get_nc()
    in_maps = []
    routing = []
    for e in range(E):
        tok, kk = np.where(order == e)
        cnt = tok.size
        assert cnt <= C, f"expert {e} overflow: {cnt} > {C}"
        xTe = np.zeros((D, C), np.float32)
        xTe[:, :cnt] = x[tok].T
        in_maps.append(
            {
                "xT": xTe,
                "W1": np.ascontiguousarray(W1[e]),
                "W2": np.ascontiguousarray(W2[e]),
                "b1": np.ascontiguousarray(b1[e].reshape(H, 1)),
            }
        )
        routing.append((tok, kk, cnt))

    kwargs = {}
    if TRACE_CORES is not None:
        kwargs["trace_cores"] = TRACE_CORES
    LAST_RESULTS = bass_utils.run_bass_kernel_spmd(
        nc, in_maps, core_ids=list(range(NCORES)), trace=TRACE, **kwargs
    )

    # --- combine: scatter-add gate-weighted expert outputs ---
    out = np.zeros((N_TOK, D), np.float32)
    for e in range(E):
        tok, kk, cnt = routing[e]
        ye = LAST_RESULTS.results[e]["y"][:cnt]
        if np.any(b2[e]):
            ye = ye + b2[e][None, :]
        # token ids are unique within one expert's list, so += is safe
        out[tok] += sw[tok, kk][:, None] * ye
    return out
